# revision 1
# baseline (speedup 1.0000x reference)
"""GAT message-passing kernel for Trainium2, 8 NeuronCores.

Problem (hardcoded): B=4, N=1024, H=F=O=G=128, E=16.
  features = concat([n_features, hidden], -1)            [B,N,256]
  values   = features @ W_m + b_m                        [B,N,128]
  logits   = att1 + att2^T + (e_features@w_ae) + att_g   [B,N,N]
  coefs    = softmax(leaky_relu(logits) + (adj-1)*1e9)
  out      = coefs @ values + features @ W_skip + b_skip

Sharding: 8 cores = (batch b = core//2) x (row half = core%2).
Each core handles 512 query rows of one batch; keys are not sharded
(the small per-batch matmuls are recomputed per core). No collectives.

Per-core on-device plan:
  - e_features (dominant, 32 MiB/core) streams as [128,512,16] half-slabs;
    the E-contraction runs as 16 fused DVE scalar_tensor_tensor MACs
    (acc = ef[:,:,e]*w_ae[e] + acc) with per-partition scalar APs.
  - att2^T (+ all scalar biases + att_g) is broadcast across partitions
    once and used as the MAC-chain seed, so logit assembly is free.
  - leaky_relu (+ att1 via the bias operand) on ScalarE (Lrelu, alpha=.01),
    exp on ScalarE.  Softmax max-subtraction is skipped (logits are O(10)
    gaussians; exp stays well inside fp32 range).
  - mask+rowsum fused in one tensor_tensor_reduce: coefs=exp*adj,
    s=rowsum (identical to adding (adj-1)*1e9 pre-softmax).
  - A@V via PE: per 128-key chunk transpose coefs on PE, copy PSUM->SBUF
    on ScalarE, matmul-accumulate against values.
  - normalization + residual fused in one STT: out = ret*(1/s) + skip.
"""

import os
import numpy as np

B, N, H, F, E, G, O = 4, 1024, 128, 128, 16, 128, 128
DIN = F + H
NCORES = 8
ROWS = N // 2          # query rows per core
RT = ROWS // 128       # row tiles per core
KC = N // 128          # key chunks
KH = 2                 # key halves for ef streaming
KHW = N // KH          # keys per half

_cache = {}


def _build(stage=4):
    from contextlib import ExitStack
    import concourse.bacc as bacc
    import concourse.tile as tile
    import concourse.mybir as mybir
    import concourse.bass as bass

    fp32 = mybir.dt.float32
    bf16 = mybir.dt.bfloat16
    ALU = mybir.AluOpType
    AF = mybir.ActivationFunctionType

    nc = bacc.Bacc("TRN2", target_bir_lowering=False, debug=False,
                   num_devices=NCORES)

    # ---- per-core I/O -------------------------------------------------
    ef_in = nc.dram_tensor("ef", [ROWS, N, E], fp32, kind="ExternalInput")
    adj_in = nc.dram_tensor("adj", [ROWS, N], fp32, kind="ExternalInput")
    nfk_in = nc.dram_tensor("nfk", [N, F], fp32, kind="ExternalInput")
    hidk_in = nc.dram_tensor("hidk", [N, H], fp32, kind="ExternalInput")
    nfr_in = nc.dram_tensor("nfr", [ROWS, F], fp32, kind="ExternalInput")
    hidr_in = nc.dram_tensor("hidr", [ROWS, H], fp32, kind="ExternalInput")
    g_in = nc.dram_tensor("g", [G, 1], fp32, kind="ExternalInput")
    Wm_in = nc.dram_tensor("Wm", [DIN, O], fp32, kind="ExternalInput")
    bm_in = nc.dram_tensor("bm", [1, O], fp32, kind="ExternalInput")
    Wsk_in = nc.dram_tensor("Wsk", [DIN, O], fp32, kind="ExternalInput")
    bsk_in = nc.dram_tensor("bsk", [1, O], fp32, kind="ExternalInput")
    wa1_in = nc.dram_tensor("wa1", [DIN, 1], fp32, kind="ExternalInput")
    wa2_in = nc.dram_tensor("wa2", [DIN, 1], fp32, kind="ExternalInput")
    wae_in = nc.dram_tensor("wae", [1, E], fp32, kind="ExternalInput")
    wag_in = nc.dram_tensor("wag", [G, 1], fp32, kind="ExternalInput")
    # b_a1 + b_a2 + b_ae + b_ag pre-summed host-side? no: pass separately
    bs_in = nc.dram_tensor("bs", [1, 4], fp32, kind="ExternalInput")
    ident_in = nc.dram_tensor("ident", [128, 128], fp32, kind="ExternalInput")
    out_t = nc.dram_tensor("out", [ROWS, O], fp32, kind="ExternalOutput")

    with tile.TileContext(nc) as tc:
        with ExitStack() as ctx:
            singles = ctx.enter_context(tc.tile_pool(name="singles", bufs=1))
            efp = ctx.enter_context(tc.tile_pool(name="efp", bufs=6))
            work = ctx.enter_context(tc.tile_pool(name="work", bufs=2))
            small = ctx.enter_context(tc.tile_pool(name="small", bufs=2))
            psT = ctx.enter_context(tc.tile_pool(name="psT", bufs=2, space="PSUM"))
            psR = ctx.enter_context(tc.tile_pool(name="psR", bufs=2, space="PSUM"))
            psS = ctx.enter_context(tc.tile_pool(name="psS", bufs=2, space="PSUM"))
            psB = ctx.enter_context(tc.tile_pool(name="psB", bufs=2, space="PSUM"))

            # ---------------- phase 0: constants & per-batch matmuls ----
            w_tile = singles.tile([128, E], bf16)       # w_ae bcast to parts
            nc.gpsimd.dma_start(out=w_tile, in_=bass.AP(
                tensor=wae_in, offset=0, ap=[[0, 128], [1, E]]))
            ident_sb = singles.tile([128, 128], fp32)
            nc.sync.dma_start(out=ident_sb, in_=ident_in.ap())
            ones_sb = singles.tile([1, 512], fp32)
            nc.vector.memset(ones_sb, 1.0)
            ones128 = ones_sb[:, :128]
            ones_bf = singles.tile([1, 512], bf16)
            nc.vector.memset(ones_bf, 1.0)
            ones128b = ones_bf[:, :128]

            Wm_sb = singles.tile([128, 2, O], bf16)
            nc.gpsimd.dma_start(out=Wm_sb, in_=Wm_in.ap().rearrange(
                "(c p) o -> p c o", p=128))
            Wsk_sb = singles.tile([128, 2, O], fp32)
            nc.sync.dma_start(out=Wsk_sb, in_=Wsk_in.ap().rearrange(
                "(c p) o -> p c o", p=128))
            wa1_sb = singles.tile([128, 2, 1], fp32)
            nc.gpsimd.dma_start(out=wa1_sb, in_=wa1_in.ap().rearrange(
                "(c p) o -> p c o", p=128))
            wa2_sb = singles.tile([128, 2, 1], bf16)
            nc.gpsimd.dma_start(out=wa2_sb, in_=wa2_in.ap().rearrange(
                "(c p) o -> p c o", p=128))
            bm_sb = singles.tile([1, O], bf16)
            nc.gpsimd.dma_start(out=bm_sb, in_=bm_in.ap())
            bsk_sb = singles.tile([1, O], fp32)
            nc.gpsimd.dma_start(out=bsk_sb, in_=bsk_in.ap())
            bs_sb = singles.tile([1, 4], fp32)
            nc.gpsimd.dma_start(out=bs_sb, in_=bs_in.ap())
            g_sb = singles.tile([128, 1], fp32)
            nc.gpsimd.dma_start(out=g_sb, in_=g_in.ap())
            wag_sb = singles.tile([128, 1], fp32)
            nc.gpsimd.dma_start(out=wag_sb, in_=wag_in.ap())

            with tc.tile_pool(name="ph0", bufs=1) as ph0:
                nfk_sb = ph0.tile([128, KC, F], fp32)
                nc.sync.dma_start(out=nfk_sb, in_=nfk_in.ap().rearrange(
                    "(c p) f -> p c f", p=128))
                hidk_sb = ph0.tile([128, KC, H], fp32)
                nc.sync.dma_start(out=hidk_sb, in_=hidk_in.ap().rearrange(
                    "(c p) f -> p c f", p=128))
                nfr_sb = ph0.tile([128, RT, F], fp32)
                nc.sync.dma_start(out=nfr_sb, in_=nfr_in.ap().rearrange(
                    "(c p) f -> p c f", p=128))
                hidr_sb = ph0.tile([128, RT, H], fp32)
                nc.sync.dma_start(out=hidr_sb, in_=hidr_in.ap().rearrange(
                    "(c p) f -> p c f", p=128))

                # features^T for keys ([f,1024] per f-chunk) and rows
                fTk0 = singles.tile([128, N], bf16)
                fTk1 = singles.tile([128, N], bf16)
                fTr0 = singles.tile([128, ROWS], fp32)
                fTr1 = singles.tile([128, ROWS], fp32)
                for kc in range(KC):
                    for src, dst in ((nfk_sb, fTk0), (hidk_sb, fTk1)):
                        tp = psT.tile([128, 128], fp32, tag="tp1")
                        nc.tensor.transpose(tp, src[:, kc, :], ident_sb)
                        nc.scalar.copy(out=dst[:, kc * 128:(kc + 1) * 128],
                                       in_=tp)
                for rc in range(RT):
                    for src, dst in ((nfr_sb, fTr0), (hidr_sb, fTr1)):
                        tp = psT.tile([128, 128], fp32, tag="tp1")
                        nc.tensor.transpose(tp, src[:, rc, :], ident_sb)
                        nc.scalar.copy(out=dst[:, rc * 128:(rc + 1) * 128],
                                       in_=tp)

                # values[k,o] per key chunk (+b_m via ones-matmul)
                V = singles.tile([128, KC, O], bf16)
                for kc in range(KC):
                    vps = psR.tile([128, O], fp32, tag="ret")
                    ksl = slice(kc * 128, (kc + 1) * 128)
                    nc.tensor.matmul(vps, fTk0[:, ksl], Wm_sb[:, 0, :],
                                     start=True, stop=False)
                    nc.tensor.matmul(vps, fTk1[:, ksl], Wm_sb[:, 1, :],
                                     start=False, stop=False)
                    nc.tensor.matmul(vps, ones128b, bm_sb,
                                     start=False, stop=True)
                    nc.scalar.copy(out=V[:, kc, :], in_=vps)

                # att1 over our rows: [128,1] per row-tile
                att1_sb = singles.tile([128, RT], fp32)
                for rc in range(RT):
                    aps = psR.tile([128, 1], fp32, tag="ret")
                    rsl = slice(rc * 128, (rc + 1) * 128)
                    nc.tensor.matmul(aps, fTr0[:, rsl], wa1_sb[:, 0, :],
                                     start=True, stop=False)
                    nc.tensor.matmul(aps, fTr1[:, rsl], wa1_sb[:, 1, :],
                                     start=False, stop=True)
                    nc.scalar.copy(out=att1_sb[:, rc:rc + 1], in_=aps)

                # att1^T rows: [1, RT*128], one 128-seg per row-tile
                att1T_sb = singles.tile([1, RT * 128], bf16)
                for rc in range(RT):
                    tpa = psT.tile([128, 128], fp32, tag="tp1")
                    nc.tensor.transpose(tpa[:1, :], att1_sb[:, rc:rc + 1],
                                        ident_sb)
                    nc.scalar.copy(out=att1T_sb[:, rc * 128:(rc + 1) * 128],
                                   in_=tpa[:1, :])

                # att2^T over all keys: [1, 1024]
                att2_sb = ph0.tile([1, N], fp32)
                for khf in range(2):
                    a2ps = psR.tile([1, 512], fp32, tag="ret")
                    ksl = slice(khf * 512, (khf + 1) * 512)
                    nc.tensor.matmul(a2ps, wa2_sb[:, 0, :], fTk0[:, ksl],
                                     start=True, stop=False)
                    nc.tensor.matmul(a2ps, wa2_sb[:, 1, :], fTk1[:, ksl],
                                     start=False, stop=True)
                    nc.scalar.copy(out=att2_sb[:, ksl], in_=a2ps)

                # att_g = g @ w_ag  (scalar), then sc = att_g + sum(biases)
                gps = psR.tile([1, 1], fp32, tag="ret")
                nc.tensor.matmul(gps, g_sb, wag_sb, start=True, stop=True)
                sc = ph0.tile([1, 1], fp32)
                nc.scalar.copy(out=sc, in_=gps)
                for i in range(4):
                    nc.vector.tensor_scalar_add(sc, sc, bs_sb[:, i:i + 1])
                att2p = singles.tile([1, N], bf16)
                nc.vector.tensor_scalar_add(att2p, att2_sb, sc)

            # ---------------- phase 1: per row-tile pipeline ------------
            for rt in range(RT if stage >= 1 else 0):
                rsl = slice(rt * 128, (rt + 1) * 128)
                adj_t = work.tile([128, N], bf16, tag="adj")
                nc.gpsimd.dma_start(out=adj_t, in_=adj_in[rsl, :])

                acc_b = work.tile([128, N], fp32, tag="acc_b")
                wta = w_tile[:]
                wpat = bass.AP(tensor=wta.tensor, offset=wta.offset,
                               ap=[list(wta.ap[0]), [0, KHW], [1, E]])
                for kh in range(KH):
                    ksl = slice(kh * KHW, (kh + 1) * KHW)
                    ef_t = efp.tile([128, KHW, E], bf16, tag="ef")
                    nc.gpsimd.dma_start(out=ef_t, in_=ef_in[rsl, ksl, :])
                    bps = psB.tile([128, KHW], fp32, tag="bps")
                    nc.tensor.matmul(bps, ones128b, att2p[:, ksl],
                                     start=True, stop=False)
                    nc.tensor.matmul(bps, att1T_sb[:, rt * 128:(rt + 1) * 128],
                                     ones_bf, start=False, stop=True)
                    wef = work.tile([128, KHW, E], bf16, tag="wef")
                    nc.vector.tensor_mul(wef, ef_t, wpat)
                    nc.vector.tensor_add(wef[:, :, 0:8], wef[:, :, 0:8],
                                         wef[:, :, 8:16])
                    nc.vector.tensor_add(wef[:, :, 0:4], wef[:, :, 0:4],
                                         wef[:, :, 4:8])
                    nc.vector.tensor_add(wef[:, :, 0:2], wef[:, :, 0:2],
                                         wef[:, :, 2:4])
                    nc.vector.tensor_add(wef[:, :, 0:1], wef[:, :, 0:1],
                                         wef[:, :, 1:2])
                    nc.vector.scalar_tensor_tensor(
                        out=acc_b[:, ksl], in0=wef[:, :, 0], scalar=1.0,
                        in1=bps, op0=ALU.mult, op1=ALU.add)
                if stage == 1:
                    o1 = work.tile([128, O], fp32, tag="outsb")
                    nc.vector.tensor_copy(o1, acc_b[:, :O])
                    nc.sync.dma_start(out=out_t[rsl, :], in_=o1)
                    continue
                lk = work.tile([128, N], fp32, tag="lk")
                if os.environ.get("GAT_SIM_LEAKY"):
                    # CoreSim lacks Lrelu; numerically identical DVE fallback
                    nc.vector.scalar_tensor_tensor(
                        out=lk, in0=acc_b, scalar=0.01, in1=acc_b,
                        op0=ALU.mult, op1=ALU.max)
                else:
                    nc.scalar.activation(lk, acc_b, AF.Lrelu, alpha=0.01)
                ex = work.tile([128, N], fp32, tag="ex")
                nc.scalar.activation(ex, lk, AF.Exp)
                coefs = work.tile([128, N], fp32, tag="coefs")
                s = small.tile([128, 1], fp32, tag="s")
                nc.vector.scalar_tensor_tensor(
                    out=coefs, in0=ex, scalar=1.0, in1=adj_t,
                    op0=ALU.mult, op1=ALU.mult, accum_out=s)
                lns = small.tile([128, 1], fp32, tag="lns")
                nc.scalar.activation(lns, s, AF.Ln)
                r = small.tile([128, 1], fp32, tag="r")
                nc.scalar.activation(r, lns, AF.Exp, scale=-1.0)
                if stage == 2:
                    o2 = work.tile([128, O], fp32, tag="outsb")
                    nc.vector.tensor_copy(o2, coefs[:, :O])
                    nc.sync.dma_start(out=out_t[rsl, :], in_=o2)
                    continue

                ret_ps = psR.tile([128, O], fp32, tag="ret")
                for kc in range(KC):
                    tp = psT.tile([128, 128], fp32, tag="tp1")
                    nc.tensor.transpose(tp, coefs[:, kc * 128:(kc + 1) * 128],
                                        ident_sb)
                    ctT = small.tile([128, 128], bf16, tag="ctT")
                    nc.scalar.copy(out=ctT, in_=tp)
                    nc.tensor.matmul(ret_ps, ctT, V[:, kc, :],
                                     start=(kc == 0), stop=(kc == KC - 1))

                if stage == 3:
                    o3 = work.tile([128, O], fp32, tag="outsb")
                    nc.vector.tensor_scalar_mul(o3, ret_ps, r)
                    nc.sync.dma_start(out=out_t[rsl, :], in_=o3)
                    continue
                sk_ps = psS.tile([128, O], fp32, tag="skp")
                nc.tensor.matmul(sk_ps, fTr0[:, rsl], Wsk_sb[:, 0, :],
                                 start=True, stop=False)
                nc.tensor.matmul(sk_ps, fTr1[:, rsl], Wsk_sb[:, 1, :],
                                 start=False, stop=False)
                nc.tensor.matmul(sk_ps, ones128, bsk_sb,
                                 start=False, stop=True)

                sk_sb = small.tile([128, O], fp32, tag="sksb")
                nc.scalar.copy(out=sk_sb, in_=sk_ps)
                out_sb = work.tile([128, O], fp32, tag="outsb")
                nc.vector.scalar_tensor_tensor(
                    out=out_sb, in0=ret_ps, scalar=r, in1=sk_sb,
                    op0=ALU.mult, op1=ALU.add)
                nc.sync.dma_start(out=out_t[rsl, :], in_=out_sb)

            if stage == 0:
                for rt in range(RT):
                    o0 = work.tile([128, O], fp32, tag="outsb")
                    nc.vector.tensor_copy(o0, V[:, rt, :])
                    nc.sync.dma_start(out=out_t[rt * 128:(rt + 1) * 128, :], in_=o0)

    nc.compile()
    return nc


def _get_nc():
    if "nc" not in _cache:
        _cache["nc"] = _build()
    return _cache["nc"]


def _in_maps(hidden, n_features, e_features, g_features, adj,
             W_m, b_m, W_skip, b_skip, w_a1, b_a1, w_a2, b_a2,
             w_ae, b_ae, w_ag, b_ag):
    f32 = np.float32
    asf = lambda x: np.ascontiguousarray(np.asarray(x, dtype=f32))
    shared = {
        "Wm": asf(W_m), "bm": asf(b_m).reshape(1, O),
        "Wsk": asf(W_skip), "bsk": asf(b_skip).reshape(1, O),
        "wa1": asf(w_a1), "wa2": asf(w_a2),
        "wae": asf(w_ae).reshape(1, E), "wag": asf(w_ag),
        "bs": np.array([[np.float32(np.asarray(b_a1).reshape(())),
                         np.float32(np.asarray(b_a2).reshape(())),
                         np.float32(np.asarray(b_ae).reshape(())),
                         np.float32(np.asarray(b_ag).reshape(()))]], dtype=f32),
        "ident": np.eye(128, dtype=f32),
    }
    maps = []
    for c in range(NCORES):
        b, h = c // 2, c % 2
        rows = slice(h * ROWS, (h + 1) * ROWS)
        m = dict(shared)
        m["ef"] = asf(e_features[b, rows])
        m["adj"] = asf(adj[b, rows])
        m["nfk"] = asf(n_features[b])
        m["hidk"] = asf(hidden[b])
        m["nfr"] = asf(n_features[b][rows])
        m["hidr"] = asf(hidden[b][rows])
        m["g"] = asf(g_features[b]).reshape(G, 1)
        maps.append(m)
    return maps


def kernel(hidden, n_features, e_features, g_features, adj,
           W_m, b_m, W_skip, b_skip, w_a1, b_a1, w_a2, b_a2,
           w_ae, b_ae, w_ag, b_ag):
    from concourse import bass_utils
    nc = _get_nc()
    maps = _in_maps(hidden, n_features, e_features, g_features, adj,
                    W_m, b_m, W_skip, b_skip, w_a1, b_a1, w_a2, b_a2,
                    w_ae, b_ae, w_ag, b_ag)
    res = bass_utils.run_bass_kernel_spmd(nc, maps, core_ids=list(range(NCORES)))
    out = np.empty((B, N, O), np.float32)
    for c in range(NCORES):
        b, h = c // 2, c % 2
        out[b, h * ROWS:(h + 1) * ROWS] = res.results[c]["out"]
    return out



# revision 3
# speedup vs baseline: 1.0428x; 1.0428x over previous
"""GAT message-passing kernel for Trainium2, 8 NeuronCores.  (v2)

Problem (hardcoded): B=4, N=1024, H=F=O=G=128, E=16.
  features = concat([n_features, hidden], -1)            [B,N,256]
  values   = features @ W_m + b_m                        [B,N,128]
  logits   = att1 + att2^T + (e_features@w_ae) + att_g   [B,N,N]
  coefs    = softmax(leaky_relu(logits) + (adj-1)*1e9)
  out      = coefs @ values + features @ W_skip + b_skip

Sharding: 8 cores = (batch b = core//2) x (row half = core%2).
Each core handles 512 query rows of one batch; keys are not sharded.

v2 strategy (v1 was DVE-bound at 81% with DMA hidden under it):
  - e_features / adj / features are cast to bf16 HOST-side (the v1 kernel
    already consumed ef as bf16 via SWDGE cast-DMA; moving the cast to the
    host halves HBM traffic and lets every DMA ride HWDGE (sync/scalar),
    freeing GpSimd from SWDGE descriptor generation).
  - adj is shipped as the additive mask (adj-1)*1e9 (bf16), so masking is
    a pre-exp add and the softmax denominator comes free via the ScalarE
    Exp(accum_out=rowsum) fused accumulator.
  - per row-tile: ONE 4 MiB ef DMA [128, N*E]; DVE runs an in-place
    mul-by-w then a pow2 tree over the e dim (bf16, 2x packed mode).
  - att2^T (+g+biases) is materialized once as a [128,N] bf16 tile;
    att1 rides the ScalarE Lrelu bias port ([128,1] per-partition).
  - A@V via PE transpose + matmul as v1; normalization via
    nc.vector.reciprocal + ScalarE scale-copy.
"""

import os
import numpy as np

B, N, H, F, E, G, O = 4, 1024, 128, 128, 16, 128, 128
DIN = F + H
NCORES = 8
ROWS = N // 2          # query rows per core
RT = ROWS // 128       # row tiles per core
KC = N // 128          # key chunks

_cache = {}


def _build(stage=4):
    from contextlib import ExitStack
    import concourse.bacc as bacc
    import concourse.tile as tile
    import concourse.mybir as mybir
    import concourse.bass as bass

    fp32 = mybir.dt.float32
    bf16 = mybir.dt.bfloat16
    ALU = mybir.AluOpType
    AF = mybir.ActivationFunctionType

    nc = bacc.Bacc("TRN2", target_bir_lowering=False, debug=False,
                   num_devices=NCORES)

    # ---- per-core I/O (all big tensors pre-cast to bf16 host-side) ----
    ef_in = nc.dram_tensor("ef", [ROWS, N, E], bf16, kind="ExternalInput")
    adjm_in = nc.dram_tensor("adjm", [ROWS, N], bf16, kind="ExternalInput")
    nfk_in = nc.dram_tensor("nfk", [N, F], bf16, kind="ExternalInput")
    hidk_in = nc.dram_tensor("hidk", [N, H], bf16, kind="ExternalInput")
    nfr_in = nc.dram_tensor("nfr", [ROWS, F], bf16, kind="ExternalInput")
    hidr_in = nc.dram_tensor("hidr", [ROWS, H], bf16, kind="ExternalInput")
    g_in = nc.dram_tensor("g", [G, 1], bf16, kind="ExternalInput")
    Wm_in = nc.dram_tensor("Wm", [DIN, O], bf16, kind="ExternalInput")
    bm_in = nc.dram_tensor("bm", [1, O], bf16, kind="ExternalInput")
    Wsk_in = nc.dram_tensor("Wsk", [DIN, O], bf16, kind="ExternalInput")
    bsk_in = nc.dram_tensor("bsk", [1, O], bf16, kind="ExternalInput")
    wa1_in = nc.dram_tensor("wa1", [DIN, 1], bf16, kind="ExternalInput")
    wa2_in = nc.dram_tensor("wa2", [DIN, 1], bf16, kind="ExternalInput")
    wbc_in = nc.dram_tensor("wbc", [128, E], bf16, kind="ExternalInput")
    wag_in = nc.dram_tensor("wag", [G, 1], bf16, kind="ExternalInput")
    bs_in = nc.dram_tensor("bs", [1, 1], fp32, kind="ExternalInput")
    ident_in = nc.dram_tensor("ident", [128, 128], bf16, kind="ExternalInput")
    out_t = nc.dram_tensor("out", [ROWS, O], fp32, kind="ExternalOutput")

    with tile.TileContext(nc) as tc:
        with ExitStack() as ctx:
            singles = ctx.enter_context(tc.tile_pool(name="singles", bufs=1))
            efp = ctx.enter_context(tc.tile_pool(name="efp", bufs=2))
            work = ctx.enter_context(tc.tile_pool(name="work", bufs=2))
            small = ctx.enter_context(tc.tile_pool(name="small", bufs=2))
            psT = ctx.enter_context(tc.tile_pool(name="psT", bufs=2, space="PSUM"))
            psR = ctx.enter_context(tc.tile_pool(name="psR", bufs=2, space="PSUM"))
            psS = ctx.enter_context(tc.tile_pool(name="psS", bufs=2, space="PSUM"))

            # ---------------- phase 0: constants & per-batch matmuls ----
            w_tile = singles.tile([128, E], bf16)       # w_ae bcast (host)
            nc.scalar.dma_start(out=w_tile, in_=wbc_in.ap())
            ident_sb = singles.tile([128, 128], bf16)
            nc.scalar.dma_start(out=ident_sb, in_=ident_in.ap())
            ones_bf = singles.tile([1, 128], bf16)
            nc.vector.memset(ones_bf, 1.0)

            Wm_sb = singles.tile([128, 2, O], bf16)
            nc.scalar.dma_start(out=Wm_sb, in_=Wm_in.ap().rearrange(
                "(c p) o -> p c o", p=128))
            Wsk_sb = singles.tile([128, 2, O], bf16)
            nc.scalar.dma_start(out=Wsk_sb, in_=Wsk_in.ap().rearrange(
                "(c p) o -> p c o", p=128))
            wa1_sb = singles.tile([128, 2, 1], bf16)
            nc.scalar.dma_start(out=wa1_sb, in_=wa1_in.ap().rearrange(
                "(c p) o -> p c o", p=128))
            wa2_sb = singles.tile([128, 2, 1], bf16)
            nc.scalar.dma_start(out=wa2_sb, in_=wa2_in.ap().rearrange(
                "(c p) o -> p c o", p=128))
            bm_sb = singles.tile([1, O], bf16)
            nc.scalar.dma_start(out=bm_sb, in_=bm_in.ap())
            bsk_sb = singles.tile([1, O], bf16)
            nc.scalar.dma_start(out=bsk_sb, in_=bsk_in.ap())
            bs_sb = singles.tile([1, 1], fp32)
            nc.scalar.dma_start(out=bs_sb, in_=bs_in.ap())
            g_sb = singles.tile([128, 1], bf16)
            nc.scalar.dma_start(out=g_sb, in_=g_in.ap())
            wag_sb = singles.tile([128, 1], bf16)
            nc.scalar.dma_start(out=wag_sb, in_=wag_in.ap())

            with tc.tile_pool(name="ph0", bufs=1) as ph0:
                nfk_sb = ph0.tile([128, KC, F], bf16)
                nc.scalar.dma_start(out=nfk_sb, in_=nfk_in.ap().rearrange(
                    "(c p) f -> p c f", p=128))
                hidk_sb = ph0.tile([128, KC, H], bf16)
                nc.scalar.dma_start(out=hidk_sb, in_=hidk_in.ap().rearrange(
                    "(c p) f -> p c f", p=128))
                nfr_sb = ph0.tile([128, RT, F], bf16)
                nc.scalar.dma_start(out=nfr_sb, in_=nfr_in.ap().rearrange(
                    "(c p) f -> p c f", p=128))
                hidr_sb = ph0.tile([128, RT, H], bf16)
                nc.scalar.dma_start(out=hidr_sb, in_=hidr_in.ap().rearrange(
                    "(c p) f -> p c f", p=128))

                # features^T for keys ([f,1024] per f-chunk) and rows
                fTk0 = singles.tile([128, N], bf16)
                fTk1 = singles.tile([128, N], bf16)
                fTr0 = singles.tile([128, ROWS], bf16)
                fTr1 = singles.tile([128, ROWS], bf16)
                for kc in range(KC):
                    for src, dst in ((nfk_sb, fTk0), (hidk_sb, fTk1)):
                        tp = psT.tile([128, 128], bf16, tag="tp1")
                        nc.tensor.transpose(tp, src[:, kc, :], ident_sb)
                        nc.scalar.copy(out=dst[:, kc * 128:(kc + 1) * 128],
                                       in_=tp)
                for rc in range(RT):
                    for src, dst in ((nfr_sb, fTr0), (hidr_sb, fTr1)):
                        tp = psT.tile([128, 128], bf16, tag="tp1")
                        nc.tensor.transpose(tp, src[:, rc, :], ident_sb)
                        nc.scalar.copy(out=dst[:, rc * 128:(rc + 1) * 128],
                                       in_=tp)

                # values[k,o] per key chunk (+b_m via ones-matmul)
                V = singles.tile([128, KC, O], bf16)
                for kc in range(KC):
                    vps = psR.tile([128, O], fp32, tag="ret")
                    ksl = slice(kc * 128, (kc + 1) * 128)
                    nc.tensor.matmul(vps, fTk0[:, ksl], Wm_sb[:, 0, :],
                                     start=True, stop=False)
                    nc.tensor.matmul(vps, fTk1[:, ksl], Wm_sb[:, 1, :],
                                     start=False, stop=False)
                    nc.tensor.matmul(vps, ones_bf, bm_sb,
                                     start=False, stop=True)
                    nc.scalar.copy(out=V[:, kc, :], in_=vps)

                # att1 over our rows: [128,1] per row-tile  (fp32 psum)
                att1_sb = singles.tile([128, RT], fp32)
                for rc in range(RT):
                    aps = psR.tile([128, 1], fp32, tag="ret")
                    rsl = slice(rc * 128, (rc + 1) * 128)
                    nc.tensor.matmul(aps, fTr0[:, rsl], wa1_sb[:, 0, :],
                                     start=True, stop=False)
                    nc.tensor.matmul(aps, fTr1[:, rsl], wa1_sb[:, 1, :],
                                     start=False, stop=True)
                    nc.scalar.copy(out=att1_sb[:, rc:rc + 1], in_=aps)

                # att_g + sum(b_a*): sc = g@wag + bs  -> bcast to [128,1]
                gps = psR.tile([1, 1], fp32, tag="ret")
                nc.tensor.matmul(gps, g_sb, wag_sb, start=True, stop=True)
                sc1 = ph0.tile([1, 1], fp32)
                nc.scalar.copy(out=sc1, in_=gps)
                nc.vector.tensor_scalar_add(sc1, sc1, bs_sb)
                sc1b = ph0.tile([1, 1], bf16)
                nc.vector.tensor_copy(sc1b, sc1)
                scps = psR.tile([128, 1], fp32, tag="ret")
                nc.tensor.matmul(scps, ones_bf, sc1b, start=True, stop=True)
                sc128 = ph0.tile([128, 1], fp32)
                nc.scalar.copy(out=sc128, in_=scps)
                # att1g = att1 + (att_g + biases): ScalarE Lrelu bias input
                att1g = singles.tile([128, RT], fp32)
                nc.vector.tensor_scalar_add(att1g, att1_sb, sc128)

                # att2 over all keys -> att2pm [128, N] bf16 (materialized)
                att2row = ph0.tile([1, N], bf16)
                for khf in range(2):
                    a2ps = psR.tile([1, 512], fp32, tag="ret")
                    ksl = slice(khf * 512, (khf + 1) * 512)
                    nc.tensor.matmul(a2ps, wa2_sb[:, 0, :], fTk0[:, ksl],
                                     start=True, stop=False)
                    nc.tensor.matmul(a2ps, wa2_sb[:, 1, :], fTk1[:, ksl],
                                     start=False, stop=True)
                    nc.scalar.copy(out=att2row[:, ksl], in_=a2ps)
                att2pm = singles.tile([128, N], bf16)
                for khf in range(2):
                    ksl = slice(khf * 512, (khf + 1) * 512)
                    bps = psS.tile([128, 512], fp32, tag="skp")
                    nc.tensor.matmul(bps, ones_bf, att2row[:, ksl],
                                     start=True, stop=True)
                    nc.scalar.copy(out=att2pm[:, ksl], in_=bps)

            # ---------------- phase 1: per row-tile pipeline ------------
            sim_leaky = bool(os.environ.get("GAT_SIM_LEAKY"))
            for rt in range(RT if stage >= 1 else 0):
                rsl = slice(rt * 128, (rt + 1) * 128)
                ef_t = efp.tile([128, N, E], bf16, tag="ef")
                nc.sync.dma_start(out=ef_t, in_=ef_in[rsl, :, :])
                adjm_t = work.tile([128, N], bf16, tag="adj")
                nc.scalar.dma_start(out=adjm_t, in_=adjm_in[rsl, :])

                wta = w_tile[:]
                wpat = bass.AP(tensor=wta.tensor, offset=wta.offset,
                               ap=[list(wta.ap[0]), [0, N], [1, E]])
                # in-place e-contraction: ef_t *= w ; pow2 tree over e
                nc.vector.tensor_mul(ef_t, ef_t, wpat)
                nc.vector.tensor_add(ef_t[:, :, 0:8], ef_t[:, :, 0:8],
                                     ef_t[:, :, 8:16])
                nc.vector.tensor_add(ef_t[:, :, 0:4], ef_t[:, :, 0:4],
                                     ef_t[:, :, 4:8])
                nc.vector.tensor_add(ef_t[:, :, 0:2], ef_t[:, :, 0:2],
                                     ef_t[:, :, 2:4])
                # last tree level -> contiguous acc_b (bf16)
                acc_b = work.tile([128, N], bf16, tag="acc_b")
                nc.vector.scalar_tensor_tensor(
                    out=acc_b, in0=ef_t[:, :, 0], scalar=0.0,
                    in1=ef_t[:, :, 1], op0=ALU.add, op1=ALU.add)
                # + att2^T(+g+b) on GpSimd (frees DVE; also probes gpsimd TT)
                acc2 = work.tile([128, N], bf16, tag="acc2")
                nc.gpsimd.tensor_add(acc2, acc_b, att2pm)
                # + additive adjacency mask
                acc3 = work.tile([128, N], bf16, tag="acc3")
                nc.vector.tensor_add(acc3, acc2, adjm_t)
                if stage == 1:
                    o1 = work.tile([128, O], fp32, tag="outsb")
                    nc.vector.tensor_copy(o1, acc3[:, :O])
                    nc.sync.dma_start(out=out_t[rsl, :], in_=o1)
                    continue

                # leaky_relu(x + att1) then exp with fused row-sum
                lk = work.tile([128, N], bf16, tag="lk")
                if sim_leaky:
                    lk2 = work.tile([128, N], bf16, tag="lk2")
                    nc.vector.tensor_scalar_add(lk2, acc3, att1g[:, rt:rt + 1])
                    nc.vector.scalar_tensor_tensor(
                        out=lk, in0=lk2, scalar=0.01, in1=lk2,
                        op0=ALU.mult, op1=ALU.max)
                else:
                    nc.scalar.activation(lk, acc3, AF.Lrelu,
                                         bias=att1g[:, rt:rt + 1], alpha=0.01)
                ex = work.tile([128, N], bf16, tag="ex")
                s = small.tile([128, 1], fp32, tag="s")
                nc.scalar.activation(ex, lk, AF.Exp, accum_out=s)
                r = small.tile([128, 1], fp32, tag="r")
                nc.vector.reciprocal(r, s)
                if stage == 2:
                    o2 = work.tile([128, O], fp32, tag="outsb")
                    nc.vector.tensor_copy(o2, ex[:, :O])
                    nc.sync.dma_start(out=out_t[rsl, :], in_=o2)
                    continue

                # A@V: transpose 128-key chunks of ex, matmul against V
                ret_ps = psR.tile([128, O], fp32, tag="ret")
                for kc in range(KC):
                    tp = psT.tile([128, 128], bf16, tag="tp1")
                    nc.tensor.transpose(tp, ex[:, kc * 128:(kc + 1) * 128],
                                        ident_sb)
                    ctT = small.tile([128, 128], bf16, tag="ctT")
                    nc.scalar.copy(out=ctT, in_=tp)
                    nc.tensor.matmul(ret_ps, ctT, V[:, kc, :],
                                     start=(kc == 0), stop=(kc == KC - 1))

                if stage == 3:
                    o3 = work.tile([128, O], fp32, tag="outsb")
                    nc.vector.tensor_scalar_mul(o3, ret_ps, r)
                    nc.sync.dma_start(out=out_t[rsl, :], in_=o3)
                    continue

                # skip connection
                sk_ps = psS.tile([128, O], fp32, tag="skp")
                nc.tensor.matmul(sk_ps, fTr0[:, rsl], Wsk_sb[:, 0, :],
                                 start=True, stop=False)
                nc.tensor.matmul(sk_ps, fTr1[:, rsl], Wsk_sb[:, 1, :],
                                 start=False, stop=False)
                nc.tensor.matmul(sk_ps, ones_bf, bsk_sb,
                                 start=False, stop=True)

                # out = ret/s + skip  (ScalarE does PSUM reads; DVE adds)
                rets = small.tile([128, O], fp32, tag="rets")
                nc.scalar.mul(rets, ret_ps, r)
                sks = small.tile([128, O], fp32, tag="sks")
                nc.scalar.copy(out=sks, in_=sk_ps)
                out_sb = work.tile([128, O], fp32, tag="outsb")
                nc.vector.tensor_add(out_sb, rets, sks)
                nc.sync.dma_start(out=out_t[rsl, :], in_=out_sb)

            if stage == 0:
                for rt in range(RT):
                    o0 = work.tile([128, O], fp32, tag="outsb")
                    nc.vector.tensor_copy(o0, V[:, rt, :])
                    nc.sync.dma_start(out=out_t[rt * 128:(rt + 1) * 128, :],
                                      in_=o0)

    nc.compile()
    return nc


def _get_nc():
    if "nc" not in _cache:
        _cache["nc"] = _build(stage=int(os.environ.get("GAT_STAGE", "4")))
    return _cache["nc"]


def _in_maps(hidden, n_features, e_features, g_features, adj,
             W_m, b_m, W_skip, b_skip, w_a1, b_a1, w_a2, b_a2,
             w_ae, b_ae, w_ag, b_ag):
    import ml_dtypes
    bf16 = ml_dtypes.bfloat16
    f32 = np.float32
    asb = lambda x: np.ascontiguousarray(np.asarray(x).astype(bf16))
    bsum = (np.float32(np.asarray(b_a1).reshape(())) +
            np.float32(np.asarray(b_a2).reshape(())) +
            np.float32(np.asarray(b_ae).reshape(())) +
            np.float32(np.asarray(b_ag).reshape(())))
    wbc = np.broadcast_to(np.asarray(w_ae, f32).reshape(1, E), (128, E))
    shared = {
        "Wm": asb(W_m), "bm": asb(b_m).reshape(1, O),
        "Wsk": asb(W_skip), "bsk": asb(b_skip).reshape(1, O),
        "wa1": asb(w_a1), "wa2": asb(w_a2),
        "wbc": asb(wbc), "wag": asb(w_ag),
        "bs": np.array([[bsum]], dtype=f32),
        "ident": np.eye(128, dtype=f32).astype(bf16),
    }
    maps = []
    adjm_all = {}
    for c in range(NCORES):
        b, h = c // 2, c % 2
        rows = slice(h * ROWS, (h + 1) * ROWS)
        if b not in adjm_all:
            adjm_all[b] = ((np.asarray(adj[b], f32) - 1.0) *
                           np.float32(1e9)).astype(bf16)
        m = dict(shared)
        m["ef"] = asb(e_features[b, rows])
        m["adjm"] = np.ascontiguousarray(adjm_all[b][rows])
        m["nfk"] = asb(n_features[b])
        m["hidk"] = asb(hidden[b])
        m["nfr"] = asb(n_features[b][rows])
        m["hidr"] = asb(hidden[b][rows])
        m["g"] = asb(g_features[b]).reshape(G, 1)
        maps.append(m)
    return maps


def kernel(hidden, n_features, e_features, g_features, adj,
           W_m, b_m, W_skip, b_skip, w_a1, b_a1, w_a2, b_a2,
           w_ae, b_ae, w_ag, b_ag):
    from concourse import bass_utils
    nc = _get_nc()
    maps = _in_maps(hidden, n_features, e_features, g_features, adj,
                    W_m, b_m, W_skip, b_skip, w_a1, b_a1, w_a2, b_a2,
                    w_ae, b_ae, w_ag, b_ag)
    res = bass_utils.run_bass_kernel_spmd(nc, maps, core_ids=list(range(NCORES)))
    out = np.empty((B, N, O), np.float32)
    for c in range(NCORES):
        b, h = c // 2, c % 2
        out[b, h * ROWS:(h + 1) * ROWS] = res.results[c]["out"]
    return out
